# revision 41
# baseline (speedup 1.0000x reference)
"""Trainium2 Bass kernel for nn_AttentionModule_39616778338491 (chord sparse attention).

Structure: V = gMLP(V); 12x { W = fMLP_m(input); V = chord_spmm(W, V) + V }.

Sharding (8 cores): core c -> batch b=c//2, half h=c%2. f-MLPs are row-split
across the pair; the g MLP is replicated over the pair (all N rows) with
host-permuted g_W2 so each core keeps exactly its chord E-half (columns 0:128
uniformly - the chord is independent per E column). The chord chain runs per
core on its own E-half; the host assembles the halves. The only collective is
one batched pair-AllGather of the 12 layers' W, issued between the f and g
TileContexts so it overlaps the g MLP.

Phase f/g: GELU evacuations are batched over chunk-pairs (same per-partition
bias column) to halve ACT op count - ACT is the MLP pacer. f second matmuls
for 4 layers run concurrently in distinct 32-wide PE column groups
(tile_position col-tiling).

Phase 2 (chord): links with offset <=128 are dense PE matmuls against two
128x128 lhsT tiles whose diagonals are rebuilt per layer by skewed flat DMAs
into DRAM staging images (diagonal writes couple partition and byte offsets,
which only DRAM-side APs allow). The 4 aligned links (256..2048) use
Y_d = w_d (x) V built by broadcast-multiply (split across DVE and GpSimd)
and accumulated via identity-lhsT matmuls of the shifted slab. Y/vbf for
layer m+1 are built per-4-block group inline with layer m's block loop
(parity-buffered) so the PE never waits a full-layer barrier; the vbf cast
runs on the Scalar engine. The +V residual is an exact-f32 vector add done
once per 4-block PSUM group.
"""

import os
import numpy as np

B, N, E, H = 4, 4096, 256, 1024
NW = 12
NL = 13
OFFS = [0, 1, 2, 4, 8, 16, 32, 64, 128, 256, 512, 1024, 2048]
ROWS = N // 2          # rows per core for f-MLP work
NBLK = N // 128        # 32 blocks of 128 rows
CH = 512               # row-chunk for MLP matmuls
NCH = ROWS // CH
HT = H // 128          # 8 h-tiles
EH = 128               # E columns per core in the chord phase
PITCH = NBLK * 128     # free width of an S tile (elems)
GROUPS = [[0, 1], [2, 3], [4, 5], [6, 7]]
NSLOT = 2              # S staging slots (src blocks +0, +1)
SLABG = 4              # V blocks per inline Y-build / residual group


def _install_patches():
    """Walrus in this image rejects >1 sem wait on the Tile tail Drain;
    spread the waits across preceding sync-engine nops. Also raise the
    stale SBUF cap (207.87 KB/partition is the real limit here)."""
    import concourse.mybir as mybir
    from concourse.tile import TileContext
    from concourse.vector_clock import ScopedClock
    from concourse import tile_utils

    def _dab(self, tick_clock, wait_clock):
        nops = [self.nc.sync.nop(nofuse=True) for _ in range(27)]
        drain_inst = self.nc.sync.drain()
        wait_clock.add_sem_waits(
            drain_inst.ins, ScopedClock({None: tick_clock.global_clock})
        )
        si = drain_inst.ins.sync_info
        waits = list(si.on_wait) if si else []
        if len(waits) > 1:
            si.on_wait.clear()
            si.on_wait.append(waits[0])
            for w, nop in zip(waits[1:], nops):
                nsi = nop.ins.sync_info
                if nsi is None:
                    nop.ins.sync_info = mybir.SyncInfo(on_update=[], on_wait=[w])
                else:
                    nsi.on_wait.append(w)
        self.nc.all_engine_barrier()
        popped = self.nc._tile_sem_poison_stack.pop()
        assert popped is self._sem_poison
        self.nc.clear_and_free_semaphores(list(self.sems.allocated().values()))
        self.nc.all_engine_barrier()

    TileContext._drain_and_barrier = _dab
    tile_utils.max_sbuf_usage = 206 * 1024


def _split_multi_waits(nc, mybir, limit=1):
    """This walrus build accepts at most one sem wait per instruction;
    hoist extra waits onto same-engine NoOps inserted just before."""
    uid = 0
    for f in nc.m.functions:
        for bb in f.blocks:
            new = []
            for inst in bb.instructions:
                si = inst.sync_info
                waits = list(si.on_wait) if si and si.on_wait else []
                if len(waits) > limit:
                    for w in waits[:-limit]:
                        nop = mybir.InstNoOp(name=f"waitsplit-{uid}", ins=[], outs=[])
                        uid += 1
                        nop.engine = inst.engine
                        nop.sync_info = mybir.SyncInfo(on_update=[], on_wait=[w])
                        new.append(nop)
                    si.on_wait.clear()
                    si.on_wait.append(waits[-1])
                new.append(inst)
            bb.instructions = new


def _build_program(nw, fp8):
    import bass_rust
    import concourse.bass as bass
    import concourse.mybir as mybir
    from concourse.tile import TileContext

    f32 = mybir.dt.float32
    bf16 = mybir.dt.bfloat16
    f8 = mybir.dt.float8e4
    DR = mybir.MatmulPerfMode.DoubleRow
    AF = mybir.ActivationFunctionType
    V64 = bass_rust.VecI64Pair

    nc = bass.Bass()
    if fp8:
        vt = nc.declare_dram_parameter("vt", [128, 2 * N], f8, isOutput=False)
        inpt = nc.declare_dram_parameter("inpt", [128, 2 * ROWS], f8, isOutput=False)
        gw1 = nc.declare_dram_parameter("gw1", [128, 2 * H], f8, isOutput=False)
        fw1 = nc.declare_dram_parameter("fw1", [nw, 128, 2 * H], f8, isOutput=False)
    else:
        vt = nc.declare_dram_parameter("vt", [E, N], bf16, isOutput=False)
        inpt = nc.declare_dram_parameter("inpt", [E, ROWS], bf16, isOutput=False)
        gw1 = nc.declare_dram_parameter("gw1", [E, H], bf16, isOutput=False)
        fw1 = nc.declare_dram_parameter("fw1", [nw, E, H], bf16, isOutput=False)
    gw2 = nc.declare_dram_parameter("gw2", [H, E], bf16, isOutput=False)
    gb1t = nc.declare_dram_parameter("gb1t", [128, HT], f32, isOutput=False)
    gb2r = nc.declare_dram_parameter("gb2r", [1, E], bf16, isOutput=False)
    fw2t = nc.declare_dram_parameter("fw2t", [nw, 128, HT * NL], bf16, isOutput=False)
    fb1t = nc.declare_dram_parameter("fb1t", [128, nw * HT], f32, isOutput=False)
    fb2c = nc.declare_dram_parameter("fb2c", [NL, nw], f32, isOutput=False)
    onesr = nc.declare_dram_parameter("onesr", [1, E], bf16, isOutput=False)
    identr = nc.declare_dram_parameter("identr", [128, 128], bf16, isOutput=False)
    out = nc.declare_dram_parameter("out", [N, EH], f32, isOutput=True)

    vloc = nc.dram_tensor("vloc", [N, EH], bf16)
    wsi_all = nc.dram_tensor("wsi_all", [nw * NL, ROWS], bf16)
    wso_all = nc.dram_tensor("wso_all", [2, nw * NL, ROWS], bf16)
    stage = [nc.dram_tensor(f"sst{p}", [NSLOT * 128 * PITCH], bf16) for p in range(2)]
    wtr = [nc.dram_tensor(f"wtr{p}", [4 * N], bf16) for p in range(2)]

    nopack = bool(int(os.environ.get("K_NOPACK", "0")))

    # ---------------- phase F: f MLPs (row-split) ----------------
    with TileContext(nc) as tc:
        with (
            tc.tile_pool(name="pc", bufs=1) as pc,
            tc.tile_pool(name="pin", bufs=1) as pin,
            tc.tile_pool(name="pfw1", bufs=2) as pfw1,
            tc.tile_pool(name="pfw2", bufs=2) as pfw2,
            tc.tile_pool(name="pfh", bufs=1) as pfh,
            tc.tile_pool(name="ptmp", bufs=4) as ptmp,
            tc.tile_pool(name="psA", bufs=3, space="PSUM") as psA,
            tc.tile_pool(name="psW", bufs=2, space="PSUM") as psW,
        ):
            fb1_t = pc.tile([128, nw * HT], f32, tag="fb1", name="fb1")
            fb2_t = pc.tile([NL, nw], f32, tag="fb2", name="fb2")
            zt = pc.tile([128, PITCH], bf16, tag="zt", name="zt")
            nc.sync.dma_start(out=fb1_t[:], in_=fb1t[:])
            nc.sync.dma_start(out=fb2_t[:], in_=fb2c[:])

            # zero the S staging images once (diagonal rewrites never touch
            # the off-diagonal zeros again)
            nc.vector.memset(zt[:], 0.0)
            for par in range(2):
                for k in range(NSLOT):
                    nc.sync.dma_start(
                        out=stage[par][k * 128 * PITCH:(k + 1) * 128 * PITCH].rearrange(
                            "(p f) -> p f", f=PITCH
                        ),
                        in_=zt[:],
                    )

            if fp8:
                inp_t = pin.tile([128, 2 * ROWS], f8, tag="inp8", name="inp8")
                nc.sync.dma_start(out=inp_t[:], in_=inpt[:])
            else:
                inp_t = [pin.tile([128, ROWS], bf16, tag=f"inp{k}", name=f"inp{k}") for k in range(2)]
                for k in range(2):
                    nc.sync.dma_start(out=inp_t[k][:], in_=inpt[k * 128:(k + 1) * 128, :])

            def first_mm_fp8(pa_sl, w1t, xt, xw, ht, c0):
                lhsT = w1t[:, ht * 128:(ht + 1) * 128]
                lhsT.ap = V64([list(lhsT.ap[0]), [H, 2], [1, 128]])
                rhs = xt[:, c0:c0 + CH]
                rhs.ap = V64([list(rhs.ap[0]), [xw, 2], [1, CH]])
                nc.tensor.matmul(pa_sl, lhsT=lhsT, rhs=rhs, start=True, stop=True,
                                 perf_mode=DR)

            m0 = 0
            while m0 < nw:
                mg = 1 if nopack else min(4, nw - m0)
                if fp8:
                    w1 = [pfw1.tile([128, 2 * H], f8, tag=f"fw18_{g}", name=f"fw18_{g}")
                          for g in range(mg)]
                    for g in range(mg):
                        nc.sync.dma_start(out=w1[g][:], in_=fw1[m0 + g])
                else:
                    w1 = [
                        [pfw1.tile([128, H], bf16, tag=f"fw1_{g}_{k}", name=f"fw1_{g}_{k}")
                         for k in range(2)]
                        for g in range(mg)
                    ]
                    for g in range(mg):
                        for k in range(2):
                            nc.sync.dma_start(
                                out=w1[g][k][:], in_=fw1[m0 + g, k * 128:(k + 1) * 128, :]
                            )
                w2 = [pfw2.tile([128, HT * NL], bf16, tag=f"fw2_{g}", name=f"fw2_{g}")
                      for g in range(mg)]
                for g in range(mg):
                    nc.sync.dma_start(out=w2[g][:], in_=fw2t[m0 + g])
                for c2 in range(NCH // 2):    # chunk-pairs: batched GELU
                    fh = [
                        [pfh.tile([128, 2 * CH], bf16, tag=f"fh{g}_{t}", name=f"fh{g}_{t}")
                         for t in range(HT)]
                        for g in range(mg)
                    ]
                    for g in range(mg):
                        m = m0 + g
                        for ht in range(HT):
                            pa = psA.tile([128, 2 * CH], f32, tag="pa", name="pa")
                            for cp in range(2):
                                c0 = (c2 * 2 + cp) * CH
                                if fp8:
                                    first_mm_fp8(pa[:, cp * CH:(cp + 1) * CH],
                                                 w1[g], inp_t, ROWS, ht, c0)
                                else:
                                    for kt in range(2):
                                        nc.tensor.matmul(
                                            pa[:, cp * CH:(cp + 1) * CH],
                                            lhsT=w1[g][kt][:, ht * 128:(ht + 1) * 128],
                                            rhs=inp_t[kt][:, c0:c0 + CH],
                                            start=(kt == 0),
                                            stop=(kt == 1),
                                        )
                            nc.scalar.activation(
                                fh[g][ht][:], pa[:], AF.Gelu,
                                bias=fb1_t[:, m * HT + ht:m * HT + ht + 1],
                            )
                    for cp in range(2):
                        pw = psW.tile([128, CH], f32, tag="pw", name="pw")
                        for ht in range(HT):
                            for g in range(mg):
                                nc.tensor.matmul(
                                    pw[32 * g:32 * g + NL, :],
                                    lhsT=w2[g][:, ht * NL:(ht + 1) * NL],
                                    rhs=fh[g][ht][:, cp * CH:(cp + 1) * CH],
                                    start=(ht == 0),
                                    stop=(ht == HT - 1),
                                    tile_position=None if nopack else (0, 32 * g),
                                )
                        for g in range(mg):
                            m = m0 + g
                            wc = ptmp.tile([NL, CH], bf16, tag=f"tw{g}", name=f"tw{g}")
                            nc.vector.tensor_scalar_add(
                                wc[:], pw[32 * g:32 * g + NL, :], fb2_t[:, m:m + 1]
                            )
                            ch = c2 * 2 + cp
                            nc.sync.dma_start(
                                out=wsi_all[m * NL:(m + 1) * NL, ch * CH:(ch + 1) * CH],
                                in_=wc[:],
                            )
                m0 += mg

    # ---------------- raw block: issue W AllGather (no wait) ----------------
    ag_sem = nc.alloc_semaphore(name="ag_sem")
    with nc.Block() as blk:
        @blk.gpsimd
        def _(g):
            g.collective_compute(
                "AllGather", mybir.AluOpType.bypass, replica_groups=GROUPS,
                ins=[wsi_all[:]], outs=[wso_all[:]],
            ).then_inc(ag_sem)

    # ---------------- phase G: g MLP (replicated rows) ----------------
    with TileContext(nc) as tc:
        with (
            tc.tile_pool(name="pcg", bufs=1) as pcg,
            tc.tile_pool(name="pvtc", bufs=2) as pvtc,
            tc.tile_pool(name="pgh", bufs=2) as pgh,
            tc.tile_pool(name="ptmpg", bufs=4) as ptmpg,
            tc.tile_pool(name="psAg", bufs=2, space="PSUM") as psAg,
            tc.tile_pool(name="psOg", bufs=4, space="PSUM") as psOg,
        ):
            gw2_t = pcg.tile([128, HT * E], bf16, tag="gw2", name="gw2")
            gb1_t = pcg.tile([128, HT], f32, tag="gb1", name="gb1")
            gb2_t = pcg.tile([1, E], bf16, tag="gb2", name="gb2")
            ones_t = pcg.tile([1, E], bf16, tag="ones", name="ones")
            for t in range(HT):
                nc.sync.dma_start(
                    out=gw2_t[:, t * E:(t + 1) * E], in_=gw2[t * 128:(t + 1) * 128, :]
                )
            nc.sync.dma_start(out=gb1_t[:], in_=gb1t[:])
            nc.sync.dma_start(out=gb2_t[:], in_=gb2r[:])
            nc.sync.dma_start(out=ones_t[:], in_=onesr[:])
            if fp8:
                gw1_t = pcg.tile([128, 2 * H], f8, tag="gw18", name="gw18")
                nc.sync.dma_start(out=gw1_t[:], in_=gw1[:])
                vt_full = pcg.tile([128, 2 * N], f8, tag="vt8", name="vt8")
                nc.sync.dma_start(out=vt_full[:], in_=vt[:])
            else:
                gw1_t = [pcg.tile([128, H], bf16, tag=f"gw1_{k}", name=f"gw1_{k}") for k in range(2)]
                for k in range(2):
                    nc.sync.dma_start(out=gw1_t[k][:], in_=gw1[k * 128:(k + 1) * 128, :])

            def gfirst_fp8(pa_sl, ht, c0):
                lhsT = gw1_t[:, ht * 128:(ht + 1) * 128]
                lhsT.ap = V64([list(lhsT.ap[0]), [H, 2], [1, 128]])
                rhs = vt_full[:, c0:c0 + CH]
                rhs.ap = V64([list(rhs.ap[0]), [N, 2], [1, CH]])
                nc.tensor.matmul(pa_sl, lhsT=lhsT, rhs=rhs, start=True, stop=True,
                                 perf_mode=DR)

            for c2 in range(N // CH // 2):
                if not fp8:
                    vt_c = [pvtc.tile([128, 2 * CH], bf16, tag=f"vtc{k}", name=f"vtc{k}")
                            for k in range(2)]
                    for k in range(2):
                        nc.sync.dma_start(
                            out=vt_c[k][:],
                            in_=vt[k * 128:(k + 1) * 128, c2 * 2 * CH:(c2 + 1) * 2 * CH],
                        )
                fh = [pgh.tile([128, 2 * CH], bf16, tag=f"gfh{t}", name=f"gfh{t}")
                      for t in range(HT)]
                for ht in range(HT):
                    pa = psAg.tile([128, 2 * CH], f32, tag="pag", name="pag")
                    for cp in range(2):
                        c0 = (c2 * 2 + cp) * CH
                        if fp8:
                            gfirst_fp8(pa[:, cp * CH:(cp + 1) * CH], ht, c0)
                        else:
                            for kt in range(2):
                                nc.tensor.matmul(
                                    pa[:, cp * CH:(cp + 1) * CH],
                                    lhsT=gw1_t[kt][:, ht * 128:(ht + 1) * 128],
                                    rhs=vt_c[kt][:, cp * CH:(cp + 1) * CH],
                                    start=(kt == 0),
                                    stop=(kt == 1),
                                )
                    nc.scalar.activation(fh[ht][:], pa[:], AF.Gelu,
                                         bias=gb1_t[:, ht:ht + 1])
                for t in range(8):
                    po = psOg.tile([128, EH], f32, tag="pog", name="pog")
                    nc.tensor.matmul(
                        po[:], lhsT=ones_t[0:1, 0:128], rhs=gb2_t[0:1, 0:EH],
                        start=True, stop=False,
                    )
                    for ht in range(HT):
                        nc.tensor.matmul(
                            po[:],
                            lhsT=fh[ht][:, t * 128:(t + 1) * 128],
                            rhs=gw2_t[:, ht * E:ht * E + EH],
                            start=False,
                            stop=(ht == HT - 1),
                        )
                    tmp = ptmpg.tile([128, EH], bf16, tag="tv", name="tv")
                    nc.vector.tensor_copy(tmp[:], po[:])
                    blk = c2 * 8 + t
                    nc.sync.dma_start(
                        out=vloc[blk * 128:(blk + 1) * 128, :], in_=tmp[:]
                    )

    # ---------------- raw block: wait for the W AllGather ----------------
    with nc.Block() as blk:
        @blk.gpsimd
        def _(g):
            g.wait_ge(ag_sem, 1)

    nc.all_engine_barrier()

    # ---------------- phase 2: chord chain (own E-half) ----------------
    with TileContext(nc) as tc:
        with (
            tc.tile_pool(name="pv", bufs=1) as pv,
            tc.tile_pool(name="ps", bufs=1) as ps,
            tc.tile_pool(name="py", bufs=1) as py,
            tc.tile_pool(name="pw", bufs=1) as pwp,
            tc.tile_pool(name="psC", bufs=4, space="PSUM") as psC,
        ):
            vfin = pv.tile([128, NBLK * EH], f32, tag="vfin", name="vfin")
            vbf = [pv.tile([128, NBLK * EH], bf16, tag=f"vbf{p}", name=f"vbf{p}")
                   for p in range(2)]
            ident_t = pv.tile([128, 128], bf16, tag="ident", name="ident")
            S = [
                [ps.tile([128, PITCH], bf16, tag=f"s{p}_{k}", name=f"s{p}_{k}")
                 for k in range(NSLOT)]
                for p in range(2)
            ]
            Y = [
                [py.tile([128, NBLK * EH], bf16, tag=f"y{p}_{d}", name=f"y{p}_{d}")
                 for d in range(4)]
                for p in range(2)
            ]
            W2t = [
                [pwp.tile([128, NBLK], bf16, tag=f"w2t{p}_{d}", name=f"w2t{p}_{d}")
                 for d in range(4)]
                for p in range(2)
            ]
            W2f = [
                [pwp.tile([128, NBLK], f32, tag=f"w2f{p}_{d}", name=f"w2f{p}_{d}")
                 for d in range(4)]
                for p in range(2)
            ]
            Wt = [pwp.tile([NL, N], bf16, tag=f"wt{p}", name=f"wt{p}") for p in range(2)]
            wt1 = [pwp.tile([NL, N], bf16, tag=f"wt1{p}", name=f"wt1{p}") for p in range(2)]

            nc.sync.dma_start(out=ident_t[:], in_=identr[:])

            def prep_w(m):
                """W-only chain for layer m: W load + interleave + S01 diag
                skew + S01 reload + W2t (rolled). Independent of V."""
                par = m % 2
                st = stage[par]
                for h2 in range(2):
                    nc.sync.dma_start(
                        out=wt1[par][:, h2 * ROWS:(h2 + 1) * ROWS],
                        in_=wso_all[h2, m * NL:(m + 1) * NL, :],
                    )
                nc.vector.tensor_copy(
                    Wt[par][:].rearrange("l (j b) -> l j b", b=NBLK),
                    wt1[par][:].rearrange("l (b j) -> l j b", j=128),
                )
                for li, d in enumerate(OFFS[:9]):
                    segs = []
                    if 128 - d > 0:
                        segs.append((0, 0, 128 - d, d))
                    if d > 0:
                        segs.append((1, 128 - d, d, 0))
                    for (si, j0, cnt, p0) in segs:
                        src = Wt[par][li:li + 1, j0 * NBLK:(j0 + cnt) * NBLK]
                        doff = si * 128 * PITCH + p0 * PITCH + j0 * NBLK
                        dst = st[doff:doff + 1]
                        dst.ap = V64([[PITCH + NBLK, cnt], [1, NBLK]])
                        nc.sync.dma_start(out=dst, in_=src)
                for k in range(NSLOT):
                    nc.sync.dma_start(
                        out=S[par][k][:],
                        in_=st[k * 128 * PITCH:(k + 1) * 128 * PITCH].rearrange(
                            "(p f) -> p f", f=PITCH
                        ),
                    )
                nc.sync.dma_start(
                    out=wtr[par][:].rearrange("(l f) -> l f", f=N), in_=Wt[par][9:13, :]
                )
                # W2t[d][j, b] = w_d[((b - dl) % 32)*128 + j]: the identity-MM
                # for out block blk reads Y slab (blk+dl), so the weight baked
                # into slab b must belong to out row (b-dl)*128+j.
                for d, dl in enumerate([2, 4, 8, 16]):
                    base = wtr[par][d * N:(d + 1) * N].rearrange("(j b) -> j b", b=NBLK)
                    nc.sync.dma_start(
                        out=W2t[par][d][:, dl:NBLK], in_=base[:, 0:NBLK - dl]
                    )
                    nc.sync.dma_start(
                        out=W2t[par][d][:, 0:dl], in_=base[:, NBLK - dl:NBLK]
                    )
                    nc.vector.tensor_copy(W2f[par][d][:], W2t[par][d][:])

            def build_y(par, b0, nb):
                """Y[par] for blocks [b0, b0+nb) from vbf[par]. Per-block
                per-partition-scalar multiplies (fast path), spread across
                DVE / GpSimd / ACT."""
                for bi in range(nb):
                    b = b0 + bi
                    c0 = b * EH
                    vv = vbf[par][:, c0:c0 + EH]
                    for d in range(4):
                        sc = W2f[par][d][:, b:b + 1]
                        yslice = Y[par][d][:, c0:c0 + EH]
                        if d < 2:
                            nc.vector.tensor_scalar_mul(yslice, vv, sc)
                        elif d == 2:
                            nc.gpsimd.tensor_scalar_mul(yslice, vv, sc)
                        else:
                            nc.scalar.activation(yslice, vv, AF.Copy, scale=sc)

            prep_w(0)
            if nw > 1:
                prep_w(1)
            # V load (bf16, my permuted E-half), per group with its Y builds
            for g0 in range(0, NBLK, SLABG):
                sv = vloc[g0 * 128:g0 * 128 + 1]
                sv.ap = V64([[EH, 128], [128 * EH, SLABG], [1, EH]])
                dvv = vbf[0][:, g0 * EH:(g0 + SLABG) * EH].rearrange(
                    "p (blk e) -> p blk e", e=EH
                )
                nc.sync.dma_start(out=dvv, in_=sv)
                build_y(0, g0, SLABG)

            for m in range(nw):
                par = m % 2
                nxt = 1 - par
                last = m == nw - 1
                for b0 in range(0, NBLK, SLABG):
                    po = psC.tile([128, SLABG * EH], f32, tag="po", name="po")
                    for bi in range(SLABG):
                        blk = b0 + bi
                        sl = po[:, bi * EH:(bi + 1) * EH]
                        nc.tensor.matmul(
                            sl, lhsT=S[par][0][:, blk::NBLK],
                            rhs=vbf[par][:, blk * EH:(blk + 1) * EH],
                            start=True, stop=False,
                        )
                        b1 = (blk + 1) % NBLK
                        nc.tensor.matmul(
                            sl, lhsT=S[par][1][:, blk::NBLK],
                            rhs=vbf[par][:, b1 * EH:(b1 + 1) * EH],
                            start=False, stop=False,
                        )
                        for i, dl in enumerate([2, 4, 8, 16]):
                            sb = (blk + dl) % NBLK
                            nc.tensor.matmul(
                                sl, lhsT=ident_t[:],
                                rhs=Y[par][i][:, sb * EH:(sb + 1) * EH],
                                start=False, stop=(i == 3),
                            )
                    # evacuate: V_next = psum (residual rides S0's diagonal)
                    c0, c1 = b0 * EH, (b0 + SLABG) * EH
                    if last:
                        nc.vector.tensor_copy(vfin[:, c0:c1], po[:])
                    else:
                        nc.vector.tensor_copy(vbf[nxt][:, c0:c1], po[:])
                        build_y(nxt, b0, SLABG)
                # next-next layer's W chain overlaps layer m+1's compute
                if m + 2 < nw:
                    prep_w(m + 2)

            if nw == 0:
                for g0 in range(0, NBLK, SLABG):
                    c0, c1 = g0 * EH, (g0 + SLABG) * EH
                    nc.vector.tensor_copy(vfin[:, c0:c1], vbf[0][:, c0:c1])
            for t in range(NBLK):
                nc.sync.dma_start(
                    out=out[t * 128:(t + 1) * 128, :], in_=vfin[:, t * EH:(t + 1) * EH]
                )

    _split_multi_waits(nc, mybir)
    return nc


def kernel(**inputs):
    _install_patches()
    from concourse.bass_utils import run_bass_kernel_spmd

    nw = int(os.environ.get("K_NW", NW))
    fp8 = bool(int(os.environ.get("K_FP8", "0")))
    V = np.ascontiguousarray(np.asarray(inputs["V"], dtype=np.float32))
    inp = np.ascontiguousarray(np.asarray(inputs["input"], dtype=np.float32))
    g_W1 = np.ascontiguousarray(np.asarray(inputs["g_W1"], dtype=np.float32))
    g_b1 = np.asarray(inputs["g_b1"], dtype=np.float32)
    g_W2 = np.ascontiguousarray(np.asarray(inputs["g_W2"], dtype=np.float32))
    g_b2 = np.asarray(inputs["g_b2"], dtype=np.float32)
    f_W1 = np.ascontiguousarray(np.asarray(inputs["f_W1"], dtype=np.float32))[:nw]
    f_b1 = np.asarray(inputs["f_b1"], dtype=np.float32)[:nw]
    f_W2 = np.ascontiguousarray(np.asarray(inputs["f_W2"], dtype=np.float32))[:nw]
    f_b2 = np.asarray(inputs["f_b2"], dtype=np.float32)[:nw]

    import ml_dtypes

    bf = ml_dtypes.bfloat16
    f8 = ml_dtypes.float8_e4m3
    gb1t = np.ascontiguousarray(g_b1.reshape(HT, 128).T)
    fw2t = np.ascontiguousarray(
        f_W2.reshape(nw, HT, 128, NL).transpose(0, 2, 1, 3).reshape(nw, 128, HT * NL)
    ).astype(bf)
    fb1t = np.ascontiguousarray(
        f_b1.reshape(nw, HT, 128).transpose(2, 0, 1).reshape(128, nw * HT)
    )
    # +1 on link 0's bias: the chord residual rides S0's diagonal, so the
    # V chain stays bf16 with one rounding per layer (~0.4-0.7% total)
    fb2c = np.ascontiguousarray(f_b2.T).copy()
    fb2c[0, :] += 1.0

    shared = {
        "gb1t": gb1t,
        "onesr": np.ones((1, E), bf),
        "identr": np.eye(128, dtype=np.float32).astype(bf),
        "fw2t": fw2t,
        "fb1t": fb1t,
        "fb2c": fb2c,
    }
    if fp8:
        shared["gw1"] = np.ascontiguousarray(
            g_W1.reshape(2, 128, H).transpose(1, 0, 2).reshape(128, 2 * H)
        ).astype(f8)
        shared["fw1"] = np.ascontiguousarray(
            f_W1.reshape(nw, 2, 128, H).transpose(0, 2, 1, 3).reshape(nw, 128, 2 * H)
        ).astype(f8)
    else:
        shared["gw1"] = g_W1.astype(bf)
        shared["fw1"] = f_W1.astype(bf)

    in_maps = []
    for c in range(8):
        b, h = c // 2, c % 2
        rows = slice(h * ROWS, (h + 1) * ROWS)
        # permute g_W2 output columns so "my chord E-half" is columns 0:EH
        eperm = np.r_[h * EH:(h + 1) * EH, (1 - h) * EH:(2 - h) * EH]
        m = dict(shared)
        m["gw2"] = np.ascontiguousarray(g_W2[:, eperm]).astype(bf)
        m["gb2r"] = np.ascontiguousarray(g_b2[eperm][None, :]).astype(bf)
        vtb = V[b].T  # [E, N]
        inb = inp[b, rows].T  # [E, ROWS]
        if fp8:
            m["vt"] = np.ascontiguousarray(
                vtb.reshape(2, 128, N).transpose(1, 0, 2).reshape(128, 2 * N)
            ).astype(f8)
            m["inpt"] = np.ascontiguousarray(
                inb.reshape(2, 128, ROWS).transpose(1, 0, 2).reshape(128, 2 * ROWS)
            ).astype(f8)
        else:
            m["vt"] = np.ascontiguousarray(vtb).astype(bf)
            m["inpt"] = np.ascontiguousarray(inb).astype(bf)
        in_maps.append(m)

    nc = _build_program(nw, fp8)
    trace = bool(int(os.environ.get("K_TRACE", "0")))
    res = run_bass_kernel_spmd(nc, in_maps, list(range(8)), trace=trace)
    kernel.last_result = res

    outp = np.empty((B, N, E), np.float32)
    for b in range(B):
        for h in range(2):
            outp[b, :, h * EH:(h + 1) * EH] = res.results[2 * b + h]["out"]
    return outp


# revision 48
# speedup vs baseline: 1.7164x; 1.7164x over previous
"""Trainium2 Bass kernel for nn_AttentionModule_39616778338491 (chord sparse attention).

Structure: V = gMLP(V); 12x { W = fMLP_m(input); V = chord_spmm(W, V) + V }.

Sharding (8 cores): core c -> batch b=c//2, half h=c%2. f-MLPs are row-split
across the pair; the g MLP is replicated over the pair (all N rows) with
host-permuted g_W2 so each core keeps exactly its chord E-half (columns 0:128
uniformly - the chord is independent per E column). The chord chain runs per
core on its own E-half; the host assembles the halves. The only collective is
one batched pair-AllGather of the 12 layers' W, issued between the f and g
TileContexts so it overlaps the g MLP.

Phase f/g: GELU evacuations are batched over chunk-pairs (same per-partition
bias column) to halve ACT op count - ACT is the MLP pacer. f second matmuls
for 4 layers run concurrently in distinct 32-wide PE column groups
(tile_position col-tiling).

Phase 2 (chord): links with offset <=128 are dense PE matmuls against two
128x128 lhsT tiles whose diagonals are rebuilt per layer by skewed flat DMAs
into DRAM staging images (diagonal writes couple partition and byte offsets,
which only DRAM-side APs allow). The 4 aligned links (256..2048) use
Y_d = w_d (x) V built by broadcast-multiply (split across DVE and GpSimd)
and accumulated via identity-lhsT matmuls of the shifted slab. Y/vbf for
layer m+1 are built per-4-block group inline with layer m's block loop
(parity-buffered) so the PE never waits a full-layer barrier; the vbf cast
runs on the Scalar engine. The +V residual is an exact-f32 vector add done
once per 4-block PSUM group.
"""

import os
import numpy as np

B, N, E, H = 4, 4096, 256, 1024
NW = 12
NL = 13
OFFS = [0, 1, 2, 4, 8, 16, 32, 64, 128, 256, 512, 1024, 2048]
ROWS = N // 2          # rows per core for f-MLP work
NBLK = N // 128        # 32 blocks of 128 rows
CH = 512               # row-chunk for MLP matmuls
NCH = ROWS // CH
HT = H // 128          # 8 h-tiles
EH = 128               # E columns per core in the chord phase
PITCH = NBLK * 128     # free width of an S tile (elems)
GROUPS = [[0, 1], [2, 3], [4, 5], [6, 7]]
NSLOT = 4              # S staging slots: src +0, +1 (links <=128), 1024, 2048
SLABG = 4              # V blocks per inline Y-build / evac group


def _install_patches():
    """Walrus in this image rejects >1 sem wait on the Tile tail Drain;
    spread the waits across preceding sync-engine nops. Also raise the
    stale SBUF cap (207.87 KB/partition is the real limit here)."""
    import concourse.mybir as mybir
    from concourse.tile import TileContext
    from concourse.vector_clock import ScopedClock
    from concourse import tile_utils

    def _dab(self, tick_clock, wait_clock):
        nops = [self.nc.sync.nop(nofuse=True) for _ in range(27)]
        drain_inst = self.nc.sync.drain()
        wait_clock.add_sem_waits(
            drain_inst.ins, ScopedClock({None: tick_clock.global_clock})
        )
        si = drain_inst.ins.sync_info
        waits = list(si.on_wait) if si else []
        if len(waits) > 1:
            si.on_wait.clear()
            si.on_wait.append(waits[0])
            for w, nop in zip(waits[1:], nops):
                nsi = nop.ins.sync_info
                if nsi is None:
                    nop.ins.sync_info = mybir.SyncInfo(on_update=[], on_wait=[w])
                else:
                    nsi.on_wait.append(w)
        self.nc.all_engine_barrier()
        popped = self.nc._tile_sem_poison_stack.pop()
        assert popped is self._sem_poison
        self.nc.clear_and_free_semaphores(list(self.sems.allocated().values()))
        self.nc.all_engine_barrier()

    TileContext._drain_and_barrier = _dab
    tile_utils.max_sbuf_usage = 206 * 1024


def _split_multi_waits(nc, mybir, limit=1):
    """This walrus build accepts at most one sem wait per instruction;
    hoist extra waits onto same-engine NoOps inserted just before."""
    uid = 0
    for f in nc.m.functions:
        for bb in f.blocks:
            new = []
            for inst in bb.instructions:
                si = inst.sync_info
                waits = list(si.on_wait) if si and si.on_wait else []
                if len(waits) > limit:
                    for w in waits[:-limit]:
                        nop = mybir.InstNoOp(name=f"waitsplit-{uid}", ins=[], outs=[])
                        uid += 1
                        nop.engine = inst.engine
                        nop.sync_info = mybir.SyncInfo(on_update=[], on_wait=[w])
                        new.append(nop)
                    si.on_wait.clear()
                    si.on_wait.append(waits[-1])
                new.append(inst)
            bb.instructions = new


def _build_program(nw, fp8):
    import bass_rust
    import concourse.bass as bass
    import concourse.mybir as mybir
    from concourse.tile import TileContext

    f32 = mybir.dt.float32
    bf16 = mybir.dt.bfloat16
    f8 = mybir.dt.float8e4
    DR = mybir.MatmulPerfMode.DoubleRow
    AF = mybir.ActivationFunctionType
    V64 = bass_rust.VecI64Pair

    nc = bass.Bass()
    if fp8:
        vt = nc.declare_dram_parameter("vt", [128, 2 * N], f8, isOutput=False)
        inpt = nc.declare_dram_parameter("inpt", [128, 2 * ROWS], f8, isOutput=False)
        gw1 = nc.declare_dram_parameter("gw1", [128, 2 * H], f8, isOutput=False)
        fw1 = nc.declare_dram_parameter("fw1", [nw, 128, 2 * H], f8, isOutput=False)
    else:
        vt = nc.declare_dram_parameter("vt", [E, N], bf16, isOutput=False)
        inpt = nc.declare_dram_parameter("inpt", [E, ROWS], bf16, isOutput=False)
        gw1 = nc.declare_dram_parameter("gw1", [E, H], bf16, isOutput=False)
        fw1 = nc.declare_dram_parameter("fw1", [nw, E, H], bf16, isOutput=False)
    gw2 = nc.declare_dram_parameter("gw2", [H, E], bf16, isOutput=False)
    gb1t = nc.declare_dram_parameter("gb1t", [128, HT], f32, isOutput=False)
    gb2r = nc.declare_dram_parameter("gb2r", [1, E], bf16, isOutput=False)
    fw2t = nc.declare_dram_parameter("fw2t", [nw, 128, HT * NL], bf16, isOutput=False)
    fb1t = nc.declare_dram_parameter("fb1t", [128, nw * HT], f32, isOutput=False)
    fb2c = nc.declare_dram_parameter("fb2c", [NL, nw], f32, isOutput=False)
    onesr = nc.declare_dram_parameter("onesr", [1, E], bf16, isOutput=False)
    identr = nc.declare_dram_parameter("identr", [128, 128], bf16, isOutput=False)
    out = nc.declare_dram_parameter("out", [N, EH], f32, isOutput=True)

    vloc = nc.dram_tensor("vloc", [N, EH], bf16)
    wsi_all = nc.dram_tensor("wsi_all", [nw * NL, ROWS], bf16)
    wso_all = nc.dram_tensor("wso_all", [2, nw * NL, ROWS], bf16)
    stage = [nc.dram_tensor(f"sst{p}", [NSLOT * 128 * PITCH], bf16) for p in range(2)]
    wtr = [nc.dram_tensor(f"wtr{p}", [2 * N], bf16) for p in range(2)]

    nopack = bool(int(os.environ.get("K_NOPACK", "0")))

    # ---------------- phase F: f MLPs (row-split) ----------------
    with TileContext(nc) as tc:
        with (
            tc.tile_pool(name="pc", bufs=1) as pc,
            tc.tile_pool(name="pin", bufs=1) as pin,
            tc.tile_pool(name="pfw1", bufs=2) as pfw1,
            tc.tile_pool(name="pfw2", bufs=2) as pfw2,
            tc.tile_pool(name="pfh", bufs=1) as pfh,
            tc.tile_pool(name="ptmp", bufs=4) as ptmp,
            tc.tile_pool(name="psA", bufs=3, space="PSUM") as psA,
            tc.tile_pool(name="psW", bufs=2, space="PSUM") as psW,
        ):
            fb1_t = pc.tile([128, nw * HT], f32, tag="fb1", name="fb1")
            fb2_t = pc.tile([NL, nw], f32, tag="fb2", name="fb2")
            zt = pc.tile([128, PITCH], bf16, tag="zt", name="zt")
            nc.sync.dma_start(out=fb1_t[:], in_=fb1t[:])
            nc.sync.dma_start(out=fb2_t[:], in_=fb2c[:])

            # zero the S staging images once (diagonal rewrites never touch
            # the off-diagonal zeros again)
            nc.vector.memset(zt[:], 0.0)
            for par in range(2):
                for k in range(NSLOT):
                    nc.sync.dma_start(
                        out=stage[par][k * 128 * PITCH:(k + 1) * 128 * PITCH].rearrange(
                            "(p f) -> p f", f=PITCH
                        ),
                        in_=zt[:],
                    )

            if fp8:
                inp_t = pin.tile([128, 2 * ROWS], f8, tag="inp8", name="inp8")
                nc.sync.dma_start(out=inp_t[:], in_=inpt[:])
            else:
                inp_t = [pin.tile([128, ROWS], bf16, tag=f"inp{k}", name=f"inp{k}") for k in range(2)]
                for k in range(2):
                    nc.sync.dma_start(out=inp_t[k][:], in_=inpt[k * 128:(k + 1) * 128, :])

            def first_mm_fp8(pa_sl, w1t, xt, xw, ht, c0):
                lhsT = w1t[:, ht * 128:(ht + 1) * 128]
                lhsT.ap = V64([list(lhsT.ap[0]), [H, 2], [1, 128]])
                rhs = xt[:, c0:c0 + CH]
                rhs.ap = V64([list(rhs.ap[0]), [xw, 2], [1, CH]])
                nc.tensor.matmul(pa_sl, lhsT=lhsT, rhs=rhs, start=True, stop=True,
                                 perf_mode=DR)

            m0 = 0
            while m0 < nw:
                mg = 1 if nopack else min(4, nw - m0)
                if fp8:
                    w1 = [pfw1.tile([128, 2 * H], f8, tag=f"fw18_{g}", name=f"fw18_{g}")
                          for g in range(mg)]
                    for g in range(mg):
                        nc.sync.dma_start(out=w1[g][:], in_=fw1[m0 + g])
                else:
                    w1 = [
                        [pfw1.tile([128, H], bf16, tag=f"fw1_{g}_{k}", name=f"fw1_{g}_{k}")
                         for k in range(2)]
                        for g in range(mg)
                    ]
                    for g in range(mg):
                        for k in range(2):
                            nc.sync.dma_start(
                                out=w1[g][k][:], in_=fw1[m0 + g, k * 128:(k + 1) * 128, :]
                            )
                w2 = [pfw2.tile([128, HT * NL], bf16, tag=f"fw2_{g}", name=f"fw2_{g}")
                      for g in range(mg)]
                for g in range(mg):
                    nc.sync.dma_start(out=w2[g][:], in_=fw2t[m0 + g])
                for c2 in range(NCH // 2):    # chunk-pairs: batched GELU
                    fh = [
                        [pfh.tile([128, 2 * CH], bf16, tag=f"fh{g}_{t}", name=f"fh{g}_{t}")
                         for t in range(HT)]
                        for g in range(mg)
                    ]
                    for g in range(mg):
                        m = m0 + g
                        for ht in range(HT):
                            pa = psA.tile([128, 2 * CH], f32, tag="pa", name="pa")
                            for cp in range(2):
                                c0 = (c2 * 2 + cp) * CH
                                if fp8:
                                    first_mm_fp8(pa[:, cp * CH:(cp + 1) * CH],
                                                 w1[g], inp_t, ROWS, ht, c0)
                                else:
                                    for kt in range(2):
                                        nc.tensor.matmul(
                                            pa[:, cp * CH:(cp + 1) * CH],
                                            lhsT=w1[g][kt][:, ht * 128:(ht + 1) * 128],
                                            rhs=inp_t[kt][:, c0:c0 + CH],
                                            start=(kt == 0),
                                            stop=(kt == 1),
                                        )
                            nc.scalar.activation(
                                fh[g][ht][:], pa[:], AF.Gelu,
                                bias=fb1_t[:, m * HT + ht:m * HT + ht + 1],
                            )
                    for cp in range(2):
                        pw = psW.tile([128, CH], f32, tag="pw", name="pw")
                        for ht in range(HT):
                            for g in range(mg):
                                nc.tensor.matmul(
                                    pw[32 * g:32 * g + NL, :],
                                    lhsT=w2[g][:, ht * NL:(ht + 1) * NL],
                                    rhs=fh[g][ht][:, cp * CH:(cp + 1) * CH],
                                    start=(ht == 0),
                                    stop=(ht == HT - 1),
                                    tile_position=None if nopack else (0, 32 * g),
                                )
                        for g in range(mg):
                            m = m0 + g
                            wc = ptmp.tile([NL, CH], bf16, tag=f"tw{g}", name=f"tw{g}")
                            nc.vector.tensor_scalar_add(
                                wc[:], pw[32 * g:32 * g + NL, :], fb2_t[:, m:m + 1]
                            )
                            ch = c2 * 2 + cp
                            nc.sync.dma_start(
                                out=wsi_all[m * NL:(m + 1) * NL, ch * CH:(ch + 1) * CH],
                                in_=wc[:],
                            )
                m0 += mg

    # ---------------- raw block: issue W AllGather (no wait) ----------------
    ag_sem = nc.alloc_semaphore(name="ag_sem")
    with nc.Block() as blk:
        @blk.gpsimd
        def _(g):
            g.collective_compute(
                "AllGather", mybir.AluOpType.bypass, replica_groups=GROUPS,
                ins=[wsi_all[:]], outs=[wso_all[:]],
            ).then_inc(ag_sem)

    # ---------------- phase G: g MLP (replicated rows) ----------------
    with TileContext(nc) as tc:
        with (
            tc.tile_pool(name="pcg", bufs=1) as pcg,
            tc.tile_pool(name="pvtc", bufs=2) as pvtc,
            tc.tile_pool(name="pgh", bufs=2) as pgh,
            tc.tile_pool(name="ptmpg", bufs=4) as ptmpg,
            tc.tile_pool(name="psAg", bufs=2, space="PSUM") as psAg,
            tc.tile_pool(name="psOg", bufs=4, space="PSUM") as psOg,
        ):
            gw2_t = pcg.tile([128, HT * E], bf16, tag="gw2", name="gw2")
            gb1_t = pcg.tile([128, HT], f32, tag="gb1", name="gb1")
            gb2_t = pcg.tile([1, E], bf16, tag="gb2", name="gb2")
            ones_t = pcg.tile([1, E], bf16, tag="ones", name="ones")
            for t in range(HT):
                nc.sync.dma_start(
                    out=gw2_t[:, t * E:(t + 1) * E], in_=gw2[t * 128:(t + 1) * 128, :]
                )
            nc.sync.dma_start(out=gb1_t[:], in_=gb1t[:])
            nc.sync.dma_start(out=gb2_t[:], in_=gb2r[:])
            nc.sync.dma_start(out=ones_t[:], in_=onesr[:])
            if fp8:
                gw1_t = pcg.tile([128, 2 * H], f8, tag="gw18", name="gw18")
                nc.sync.dma_start(out=gw1_t[:], in_=gw1[:])
                vt_full = pcg.tile([128, 2 * N], f8, tag="vt8", name="vt8")
                nc.sync.dma_start(out=vt_full[:], in_=vt[:])
            else:
                gw1_t = [pcg.tile([128, H], bf16, tag=f"gw1_{k}", name=f"gw1_{k}") for k in range(2)]
                for k in range(2):
                    nc.sync.dma_start(out=gw1_t[k][:], in_=gw1[k * 128:(k + 1) * 128, :])

            def gfirst_fp8(pa_sl, ht, c0):
                lhsT = gw1_t[:, ht * 128:(ht + 1) * 128]
                lhsT.ap = V64([list(lhsT.ap[0]), [H, 2], [1, 128]])
                rhs = vt_full[:, c0:c0 + CH]
                rhs.ap = V64([list(rhs.ap[0]), [N, 2], [1, CH]])
                nc.tensor.matmul(pa_sl, lhsT=lhsT, rhs=rhs, start=True, stop=True,
                                 perf_mode=DR)

            for c2 in range(N // CH // 2):
                if not fp8:
                    vt_c = [pvtc.tile([128, 2 * CH], bf16, tag=f"vtc{k}", name=f"vtc{k}")
                            for k in range(2)]
                    for k in range(2):
                        nc.sync.dma_start(
                            out=vt_c[k][:],
                            in_=vt[k * 128:(k + 1) * 128, c2 * 2 * CH:(c2 + 1) * 2 * CH],
                        )
                fh = [pgh.tile([128, 2 * CH], bf16, tag=f"gfh{t}", name=f"gfh{t}")
                      for t in range(HT)]
                for ht in range(HT):
                    pa = psAg.tile([128, 2 * CH], f32, tag="pag", name="pag")
                    for cp in range(2):
                        c0 = (c2 * 2 + cp) * CH
                        if fp8:
                            gfirst_fp8(pa[:, cp * CH:(cp + 1) * CH], ht, c0)
                        else:
                            for kt in range(2):
                                nc.tensor.matmul(
                                    pa[:, cp * CH:(cp + 1) * CH],
                                    lhsT=gw1_t[kt][:, ht * 128:(ht + 1) * 128],
                                    rhs=vt_c[kt][:, cp * CH:(cp + 1) * CH],
                                    start=(kt == 0),
                                    stop=(kt == 1),
                                )
                    nc.scalar.activation(fh[ht][:], pa[:], AF.Gelu,
                                         bias=gb1_t[:, ht:ht + 1])
                for t in range(8):
                    po = psOg.tile([128, EH], f32, tag="pog", name="pog")
                    nc.tensor.matmul(
                        po[:], lhsT=ones_t[0:1, 0:128], rhs=gb2_t[0:1, 0:EH],
                        start=True, stop=False,
                    )
                    for ht in range(HT):
                        nc.tensor.matmul(
                            po[:],
                            lhsT=fh[ht][:, t * 128:(t + 1) * 128],
                            rhs=gw2_t[:, ht * E:ht * E + EH],
                            start=False,
                            stop=(ht == HT - 1),
                        )
                    tmp = ptmpg.tile([128, EH], bf16, tag="tv", name="tv")
                    nc.vector.tensor_copy(tmp[:], po[:])
                    blk = c2 * 8 + t
                    nc.sync.dma_start(
                        out=vloc[blk * 128:(blk + 1) * 128, :], in_=tmp[:]
                    )

    # ---------------- raw block: wait for the W AllGather ----------------
    with nc.Block() as blk:
        @blk.gpsimd
        def _(g):
            g.wait_ge(ag_sem, 1)

    nc.all_engine_barrier()

    # ---------------- phase 2: chord chain (own E-half) ----------------
    with TileContext(nc) as tc:
        with (
            tc.tile_pool(name="pv", bufs=1) as pv,
            tc.tile_pool(name="ps", bufs=1) as ps,
            tc.tile_pool(name="py", bufs=1) as py,
            tc.tile_pool(name="pw", bufs=1) as pwp,
            tc.tile_pool(name="psC", bufs=4, space="PSUM") as psC,
        ):
            vfin = pv.tile([128, NBLK * EH], f32, tag="vfin", name="vfin")
            vbf = [pv.tile([128, NBLK * EH], bf16, tag=f"vbf{p}", name=f"vbf{p}")
                   for p in range(2)]
            ident_t = pv.tile([128, 128], bf16, tag="ident", name="ident")
            S = [
                [ps.tile([128, PITCH], bf16, tag=f"s{p}_{k}", name=f"s{p}_{k}")
                 for k in range(NSLOT)]
                for p in range(2)
            ]
            Y = [
                [py.tile([128, NBLK * EH], bf16, tag=f"y{p}_{d}", name=f"y{p}_{d}")
                 for d in range(2)]
                for p in range(2)
            ]
            W2t = [
                [pwp.tile([128, NBLK], bf16, tag=f"w2t{p}_{d}", name=f"w2t{p}_{d}")
                 for d in range(2)]
                for p in range(2)
            ]
            Wt = [pwp.tile([NL, N], bf16, tag=f"wt{p}", name=f"wt{p}") for p in range(2)]
            wt1 = [pwp.tile([NL, N], bf16, tag=f"wt1{p}", name=f"wt1{p}") for p in range(2)]

            nc.sync.dma_start(out=ident_t[:], in_=identr[:])

            def prep_w(m):
                """W-only chain for layer m: W load + interleave + S01 diag
                skew + S01 reload + W2t (rolled). Independent of V."""
                par = m % 2
                st = stage[par]
                for h2 in range(2):
                    nc.sync.dma_start(
                        out=wt1[par][:, h2 * ROWS:(h2 + 1) * ROWS],
                        in_=wso_all[h2, m * NL:(m + 1) * NL, :],
                    )
                nc.vector.tensor_copy(
                    Wt[par][:].rearrange("l (j b) -> l j b", b=NBLK),
                    wt1[par][:].rearrange("l (b j) -> l j b", j=128),
                )
                for li, d in enumerate(OFFS):
                    if d <= 128:
                        segs = []
                        if 128 - d > 0:
                            segs.append((0, 0, 128 - d, d))
                        if d > 0:
                            segs.append((1, 128 - d, d, 0))
                    elif d in (1024, 2048):
                        segs = [({1024: 2, 2048: 3}[d], 0, 128, 0)]
                    else:
                        continue   # 256/512 go through Y
                    for (si, j0, cnt, p0) in segs:
                        src = Wt[par][li:li + 1, j0 * NBLK:(j0 + cnt) * NBLK]
                        doff = si * 128 * PITCH + p0 * PITCH + j0 * NBLK
                        dst = st[doff:doff + 1]
                        dst.ap = V64([[PITCH + NBLK, cnt], [1, NBLK]])
                        nc.sync.dma_start(out=dst, in_=src)
                for k in range(NSLOT):
                    nc.sync.dma_start(
                        out=S[par][k][:],
                        in_=st[k * 128 * PITCH:(k + 1) * 128 * PITCH].rearrange(
                            "(p f) -> p f", f=PITCH
                        ),
                    )
                nc.sync.dma_start(
                    out=wtr[par][:].rearrange("(l f) -> l f", f=N), in_=Wt[par][9:11, :]
                )
                # W2t[d][j, b] = w_d[((b - dl) % 32)*128 + j]: the identity-MM
                # for out block blk reads Y slab (blk+dl), so the weight baked
                # into slab b must belong to out row (b-dl)*128+j.
                for d, dl in enumerate([2, 4]):
                    base = wtr[par][d * N:(d + 1) * N].rearrange("(j b) -> j b", b=NBLK)
                    nc.sync.dma_start(
                        out=W2t[par][d][:, dl:NBLK], in_=base[:, 0:NBLK - dl]
                    )
                    nc.sync.dma_start(
                        out=W2t[par][d][:, 0:dl], in_=base[:, NBLK - dl:NBLK]
                    )

            def build_y(par, b0, nb):
                """Y[par] (offsets 256/512) for blocks [b0, b0+nb) from
                vbf[par]; one broadcast multiply per offset, DVE + GpSimd."""
                c0, c1 = b0 * EH, (b0 + nb) * EH
                for d in range(2):
                    vv = vbf[par][:, c0:c1].rearrange("p (blk e) -> p blk e", e=EH)
                    ww = W2t[par][d][:, b0:b0 + nb]
                    ww.ap = V64([list(ww.ap[0]), [1, nb], [0, EH]])
                    yy = Y[par][d][:, c0:c1].rearrange("p (blk e) -> p blk e", e=EH)
                    eng = nc.vector if d == 0 else nc.gpsimd
                    eng.tensor_mul(yy, vv, ww)

            prep_w(0)
            if nw > 1:
                prep_w(1)
            # V load (bf16, my permuted E-half), per group with its Y builds
            for g0 in range(0, NBLK, SLABG):
                sv = vloc[g0 * 128:g0 * 128 + 1]
                sv.ap = V64([[EH, 128], [128 * EH, SLABG], [1, EH]])
                dvv = vbf[0][:, g0 * EH:(g0 + SLABG) * EH].rearrange(
                    "p (blk e) -> p blk e", e=EH
                )
                nc.sync.dma_start(out=dvv, in_=sv)
                build_y(0, g0, SLABG)

            for m in range(nw):
                par = m % 2
                nxt = 1 - par
                last = m == nw - 1
                for b0 in range(0, NBLK, SLABG):
                    po = psC.tile([128, SLABG * EH], f32, tag="po", name="po")
                    for bi in range(SLABG):
                        blk = b0 + bi
                        sl = po[:, bi * EH:(bi + 1) * EH]
                        nc.tensor.matmul(
                            sl, lhsT=S[par][0][:, blk::NBLK],
                            rhs=vbf[par][:, blk * EH:(blk + 1) * EH],
                            start=True, stop=False,
                        )
                        for k, dl in ((1, 1), (2, 8), (3, 16)):
                            sb = (blk + dl) % NBLK
                            nc.tensor.matmul(
                                sl, lhsT=S[par][k][:, blk::NBLK],
                                rhs=vbf[par][:, sb * EH:(sb + 1) * EH],
                                start=False, stop=False,
                            )
                        for i, dl in enumerate([2, 4]):
                            sb = (blk + dl) % NBLK
                            nc.tensor.matmul(
                                sl, lhsT=ident_t[:],
                                rhs=Y[par][i][:, sb * EH:(sb + 1) * EH],
                                start=False, stop=(i == 1),
                            )
                    # evacuate: V_next = psum (residual rides S0's diagonal)
                    c0, c1 = b0 * EH, (b0 + SLABG) * EH
                    if last:
                        nc.vector.tensor_copy(vfin[:, c0:c1], po[:])
                    else:
                        nc.vector.tensor_copy(vbf[nxt][:, c0:c1], po[:])
                        build_y(nxt, b0, SLABG)
                # next-next layer's W chain overlaps layer m+1's compute
                if m + 2 < nw:
                    prep_w(m + 2)

            if nw == 0:
                for g0 in range(0, NBLK, SLABG):
                    c0, c1 = g0 * EH, (g0 + SLABG) * EH
                    nc.vector.tensor_copy(vfin[:, c0:c1], vbf[0][:, c0:c1])
            for t in range(NBLK):
                nc.sync.dma_start(
                    out=out[t * 128:(t + 1) * 128, :], in_=vfin[:, t * EH:(t + 1) * EH]
                )

    _split_multi_waits(nc, mybir)
    return nc


def kernel(**inputs):
    _install_patches()
    from concourse.bass_utils import run_bass_kernel_spmd

    nw = int(os.environ.get("K_NW", NW))
    fp8 = bool(int(os.environ.get("K_FP8", "0")))
    V = np.ascontiguousarray(np.asarray(inputs["V"], dtype=np.float32))
    inp = np.ascontiguousarray(np.asarray(inputs["input"], dtype=np.float32))
    g_W1 = np.ascontiguousarray(np.asarray(inputs["g_W1"], dtype=np.float32))
    g_b1 = np.asarray(inputs["g_b1"], dtype=np.float32)
    g_W2 = np.ascontiguousarray(np.asarray(inputs["g_W2"], dtype=np.float32))
    g_b2 = np.asarray(inputs["g_b2"], dtype=np.float32)
    f_W1 = np.ascontiguousarray(np.asarray(inputs["f_W1"], dtype=np.float32))[:nw]
    f_b1 = np.asarray(inputs["f_b1"], dtype=np.float32)[:nw]
    f_W2 = np.ascontiguousarray(np.asarray(inputs["f_W2"], dtype=np.float32))[:nw]
    f_b2 = np.asarray(inputs["f_b2"], dtype=np.float32)[:nw]

    import ml_dtypes

    bf = ml_dtypes.bfloat16
    f8 = ml_dtypes.float8_e4m3
    gb1t = np.ascontiguousarray(g_b1.reshape(HT, 128).T)
    fw2t = np.ascontiguousarray(
        f_W2.reshape(nw, HT, 128, NL).transpose(0, 2, 1, 3).reshape(nw, 128, HT * NL)
    ).astype(bf)
    fb1t = np.ascontiguousarray(
        f_b1.reshape(nw, HT, 128).transpose(2, 0, 1).reshape(128, nw * HT)
    )
    # +1 on link 0's bias: the chord residual rides S0's diagonal, so the
    # V chain stays bf16 with one rounding per layer (~0.4-0.7% total)
    fb2c = np.ascontiguousarray(f_b2.T).copy()
    fb2c[0, :] += 1.0

    shared = {
        "gb1t": gb1t,
        "onesr": np.ones((1, E), bf),
        "identr": np.eye(128, dtype=np.float32).astype(bf),
        "fw2t": fw2t,
        "fb1t": fb1t,
        "fb2c": fb2c,
    }
    if fp8:
        shared["gw1"] = np.ascontiguousarray(
            g_W1.reshape(2, 128, H).transpose(1, 0, 2).reshape(128, 2 * H)
        ).astype(f8)
        shared["fw1"] = np.ascontiguousarray(
            f_W1.reshape(nw, 2, 128, H).transpose(0, 2, 1, 3).reshape(nw, 128, 2 * H)
        ).astype(f8)
    else:
        shared["gw1"] = g_W1.astype(bf)
        shared["fw1"] = f_W1.astype(bf)

    in_maps = []
    for c in range(8):
        b, h = c // 2, c % 2
        rows = slice(h * ROWS, (h + 1) * ROWS)
        # permute g_W2 output columns so "my chord E-half" is columns 0:EH
        eperm = np.r_[h * EH:(h + 1) * EH, (1 - h) * EH:(2 - h) * EH]
        m = dict(shared)
        m["gw2"] = np.ascontiguousarray(g_W2[:, eperm]).astype(bf)
        m["gb2r"] = np.ascontiguousarray(g_b2[eperm][None, :]).astype(bf)
        vtb = V[b].T  # [E, N]
        inb = inp[b, rows].T  # [E, ROWS]
        if fp8:
            m["vt"] = np.ascontiguousarray(
                vtb.reshape(2, 128, N).transpose(1, 0, 2).reshape(128, 2 * N)
            ).astype(f8)
            m["inpt"] = np.ascontiguousarray(
                inb.reshape(2, 128, ROWS).transpose(1, 0, 2).reshape(128, 2 * ROWS)
            ).astype(f8)
        else:
            m["vt"] = np.ascontiguousarray(vtb).astype(bf)
            m["inpt"] = np.ascontiguousarray(inb).astype(bf)
        in_maps.append(m)

    nc = _build_program(nw, fp8)
    trace = bool(int(os.environ.get("K_TRACE", "0")))
    res = run_bass_kernel_spmd(nc, in_maps, list(range(8)), trace=trace)
    kernel.last_result = res

    outp = np.empty((B, N, E), np.float32)
    for b in range(B):
        for h in range(2):
            outp[b, :, h * EH:(h + 1) * EH] = res.results[2 * b + h]["out"]
    return outp
